# revision 41
# baseline (speedup 1.0000x reference)
"""MinkowskiFlow coarse-flow kernel for 8 Trainium2 NeuronCores (Bass/Tile).

Math (per batch b):
    fs = normalize(feat_s); ft = normalize(feat_t)
    C[n,m]   = 2 - 2 <fs_n, ft_m>
    K[n,m]   = exp(-C/(exp(eps)+0.03)) * (||coor_s_n - coor_t_m||^2 < 100)
    out[n,:] = (K @ coor_t) / (sum_m K + 1e-8) - coor_s

Sharding: batch b -> 4 cores each (data parallel over B=2), N split into 4
row blocks of 1024 (row-wise; each row's normalization is independent).

Per-core design (target index m on SBUF partitions, all operands built by
PE transpose-through-identity -> PSUM -> ACT copy; no DMA-xbar transposes):

  layout     All HBM<->SBUF transfers use the permuted order row(p,t)=p*T+t
             so each partition reads/writes one contiguous block (single
             descriptor) per DMA.  The row permutation is absorbed: every
             per-tile consumer (ftT tile, srn/thr column, ct36 block, the
             agg reduction over m) indexes tiles consistently, and the
             output store inverts the n permutation; out is staged in SBUF
             and written with ONE DMA per rep.
  S^T[m,n]   single bf16 PE matmul (K=64) of RAW bf16 ft rows against
             normalized bf16 fs columns; the ft row normalization is folded
             into exp's per-partition scale AP (srn = (2/tau)/||ft_m||), so
             the ft transposes depend only on the DMA, not the norm chain.
  dist mask  R'[m,n] = -2*ctc_m . csc_n + |csc_n|^2 as ONE K=21 bf16 matmul
             over a 3-way bf16 split (h+m+l) of CENTERED coords: terms
             h.h, cs2(h,m,l), h.m, m.h, m.m, h.l, l.h in that accumulation
             order (large terms first; knife-edge radius pairs need ~1e-4).
             mask = R' < 100 - |ctc_m|^2  (fp32 threshold per partition)
  K^T        = exp(srn_m * S_raw^T - 2/tau) * mask: one ACT op (bf16 out,
             per-partition scale) + one fused DVE scalar_tensor_tensor
             (is_lt, mult), bf16.
  agg        ONE bf16 matmul per tile: lhsT [128,36] holds [ct_hi | 1] in
             cols 0:4 and [ct_lo | 0] in cols 32:36 -> PSUM [36, n];
             rows 0:4 + rows 32:36 added at the end (exact coord split).
             Issued with a 3-iteration software-pipeline lag so the PE
             never waits on the exp/mask chain.
  engines    Pool(gpsimd) runs the SBUF-only prep (coord splits, squares,
             builder fills) in parallel with DVE; Pool cannot touch PSUM
             or run TensorScalarPtr (AP-scalar) ops on this target, so the
             mask ops and PSUM drains sit on DVE/ACT.  PSUM banks: psA 2 +
             psB 2 + psG 2 + psT 2 = 8.
Final per chunk: 4 PE transposes of the agg rows into one PSUM tile, one
DVE copy, vectorized reciprocal, out = acc*recip(rs+1e-8) - coor_s.

build_kernel(loop_r=R) wraps the identical per-rep body in a For_i
hardware loop for test.py's amortized device-time measurement (the For_i
back-edge barrier measures ~0 ns on HW).
"""
import numpy as np
from contextlib import ExitStack

import concourse.bass as bass
import concourse.bacc as bacc
import concourse.tile as tile
import concourse.mybir as mybir
from concourse import masks
from concourse.bass_utils import run_bass_kernel_spmd

F32 = mybir.dt.float32
BF16 = mybir.dt.bfloat16
AF = mybir.ActivationFunctionType
ALU = mybir.AluOpType

B, N, M, D = 2, 4096, 4096, 64
N_CORES = 8
CORES_PER_BATCH = N_CORES // B      # 4
NS = N // CORES_PER_BATCH           # 1024 source rows per core
P = 128
MT = M // P                         # 32 target tiles
NT = NS // P                        # 8 source tiles per core
CHUNK = 512
NCHUNK = NS // CHUNK                # 2
CENTER = 20.0
TAU_OFFSET = 0.03
RADIUS_SQ = 100.0
KC = 21                             # coord-matmul contraction rows
KW = 32                             # coord builder width (KC rounded up)


def build_kernel(tau: float, repeat: int = 1, loop_r: int | None = None,
                 diag: str | None = None):
    nc = bacc.Bacc("TRN2", target_bir_lowering=False, debug=False,
                   num_devices=N_CORES)
    fs_d = nc.dram_tensor("fs", [NS, D], F32, kind="ExternalInput").ap()
    ft_d = nc.dram_tensor("ft", [M, D], F32, kind="ExternalInput").ap()
    cs_d = nc.dram_tensor("cs", [NS, 3], F32, kind="ExternalInput").ap()
    ct_d = nc.dram_tensor("ct", [M, 3], F32, kind="ExternalInput").ap()
    out_d = nc.dram_tensor("out", [NS, 3], F32, kind="ExternalOutput").ap()

    scale = float(2.0 / tau)

    with tile.TileContext(nc) as tc, ExitStack() as ctx:
        pers = ctx.enter_context(tc.tile_pool(name="pers", bufs=1))
        scr = ctx.enter_context(tc.tile_pool(name="scr", bufs=3))
        sbE = ctx.enter_context(tc.tile_pool(name="sbE", bufs=19))
        sbK = ctx.enter_context(tc.tile_pool(name="sbK", bufs=8))
        fin = ctx.enter_context(tc.tile_pool(name="fin", bufs=2))
        psA = ctx.enter_context(tc.tile_pool(name="psA", bufs=2, space="PSUM"))
        psB = ctx.enter_context(tc.tile_pool(name="psB", bufs=2, space="PSUM"))
        psG = ctx.enter_context(tc.tile_pool(name="psG", bufs=2, space="PSUM"))
        psT = ctx.enter_context(tc.tile_pool(name="psT", bufs=2, space="PSUM"))

        # ---------------- persistent tensors ----------------
        ftT = pers.tile([D, M], BF16)       # ft_norm^T (d on partitions)
        rhsA = pers.tile([D, NS], BF16)     # fs_norm^T
        lhsC = pers.tile([KW, M], BF16)     # coord lhsT rows 0:KC (see header)
        rhsC = pers.tile([KW, NS], BF16)    # coord rhs rows 0:KC
        thr = pers.tile([P, MT], F32)       # 100 - |ct-20|^2 per m-tile column
        ct36 = pers.tile([P, 36 * MT], BF16)  # agg lhsT: [ct_hi|1] , [ct_lo|0]
        ident = pers.tile([P, P], F32)
        identB = pers.tile([P, P], BF16)
        biasT = pers.tile([P, 1], F32)

        ft_all = pers.tile([P, MT * D], F32)
        fs_all = pers.tile([P, NT * D], F32)
        ct_all = pers.tile([P, MT * 3], F32)
        cs_all = pers.tile([P, NT * 3], F32)
        s2t = pers.tile([P, MT], F32)
        s2s = pers.tile([P, NT], F32)
        ct2c = pers.tile([P, MT], F32)
        cs2c = pers.tile([P, NT], F32)
        fnt = pers.tile([P, MT * D], BF16)   # normalized ft, bf16
        fns = pers.tile([P, NT * D], BF16)   # normalized fs, bf16
        # coord splits (target / source), 3 cols per tile
        ctn_all = pers.tile([P, MT * 3], F32)
        th_all = pers.tile([P, MT * 3], BF16)
        tm_all = pers.tile([P, MT * 3], BF16)
        tl_all = pers.tile([P, MT * 3], BF16)
        tr1 = pers.tile([P, MT * 3], F32)
        csc_all = pers.tile([P, NT * 3], F32)
        sh_all = pers.tile([P, NT * 3], BF16)
        sm_all = pers.tile([P, NT * 3], BF16)
        sl_all = pers.tile([P, NT * 3], BF16)
        sr1 = pers.tile([P, NT * 3], F32)
        c2h = pers.tile([P, NT], BF16)
        c2m = pers.tile([P, NT], BF16)
        c2l = pers.tile([P, NT], BF16)
        c2r = pers.tile([P, NT], F32)
        cth_all = pers.tile([P, MT * 3], BF16)
        ctl_all = pers.tile([P, MT * 3], BF16)
        rbt = pers.tile([P, MT * KW], BF16)  # row-layout coord lhsT builder
        rbs = pers.tile([P, NT * KW], BF16)  # (cols KC:KW zero-padded)
        out_sb = pers.tile([P, NT * 3], F32)
        srn = pers.tile([P, MT], F32)   # scale / ||ft_row||, per m-tile column

        masks.make_identity(nc, ident[:])
        masks.make_identity(nc, identB[:])
        nc.vector.memset(biasT[:], -scale)
        # rep-invariant padding of builder tiles (zeros/ones persist)
        nc.vector.memset(rbt[:], 0.0)
        nc.vector.memset(rbs[:], 0.0)
        nc.vector.memset(ct36[:], 0.0)
        v36i = ct36[:].rearrange("p (t k) -> p t k", k=36)
        nc.vector.memset(v36i[:, :, 3:4], 1.0)
        rti = rbt[:].rearrange("p (t k) -> p t k", k=KW)
        nc.vector.memset(rti[:, :, 3:6], 1.0)

        def _body():
            if diag == 'nop':
                nc.vector.memset(biasT[:], -scale)
                return
            # ---------------- load inputs ----------------
            # Permuted on-chip order: tile t holds rows {p*T + t} (p = SBUF
            # partition), so every DRAM access is contiguous per partition
            # (one big descriptor each) instead of a 256B gather.  The m
            # permutation is absorbed: all per-tile consumers (ftT tile,
            # thr column, ct36 block, agg reduction over m) index tiles
            # consistently, and the n permutation is inverted by the output
            # store's access pattern.
            ftv_l = ft_all[:].rearrange("p (t d) -> p t d", d=D)
            ftd_l = ft_d.rearrange("(p t) d -> p t d", t=MT)
            nc.sync.dma_start(ftv_l[0:P // 2], ftd_l[0:P // 2])
            nc.scalar.dma_start(ftv_l[P // 2:P], ftd_l[P // 2:P])
            nc.scalar.dma_start(
                fs_all[:].rearrange("p (t d) -> p t d", d=D),
                fs_d.rearrange("(p t) d -> p t d", t=NT))
            nc.sync.dma_start(
                ct_all[:].rearrange("p (t c) -> p t c", c=3),
                ct_d.rearrange("(p t) c -> p t c", t=MT))
            nc.scalar.dma_start(
                cs_all[:].rearrange("p (t c) -> p t c", c=3),
                cs_d.rearrange("(p t) c -> p t c", t=NT))

            if diag == 'dma':
                return
            ftv = ft_all[:].rearrange("p (t d) -> p t d", d=D)
            fsv = fs_all[:].rearrange("p (t d) -> p t d", d=D)
            csv = cs_all[:].rearrange("p (t c) -> p t c", c=3)

            # ------------- feature normalization -------------
            # ft side: do NOT scale the features; exp() applies the row
            # normalization via its per-partition scale AP (srn = scale/||ft||).
            # So fnt is a plain bf16 copy of raw ft and the ft transposes
            # depend only on the DMA, not on the norm chain.
            nc.gpsimd.tensor_copy(fnt[:], ft_all[:])
            sqf = scr.tile([P, MT * D], F32, tag="sqf")
            nc.gpsimd.tensor_tensor(sqf[:], ft_all[:], ft_all[:], op=ALU.mult)
            nc.vector.tensor_reduce(
                s2t[:], sqf[:].rearrange("p (t d) -> p t d", d=D),
                axis=mybir.AxisListType.X, op=ALU.add)
            nc.scalar.sqrt(srn[:], s2t[:])
            nc.vector.reciprocal(srn[:], srn[:])
            nc.vector.tensor_scalar(srn[:], srn[:], scale, 0.0,
                                    op0=ALU.mult, op1=ALU.add)
            # fs side: normalized bf16 (rhs columns need true unit rows)
            fnsv = fns[:].rearrange("p (t d) -> p t d", d=D)
            sqs2 = scr.tile([P, NT * D], F32, tag="sqs2")
            nc.gpsimd.tensor_tensor(sqs2[:], fs_all[:], fs_all[:], op=ALU.mult)
            nc.vector.tensor_reduce(
                s2s[:], sqs2[:].rearrange("p (t d) -> p t d", d=D),
                axis=mybir.AxisListType.X, op=ALU.add)
            rns = scr.tile([P, NT], F32, tag="rn")
            nc.scalar.sqrt(rns[:], s2s[:])
            nc.vector.reciprocal(rns[:], rns[:])
            for t in range(NT):
                nc.vector.tensor_scalar_mul(fnsv[:, t, :], fsv[:, t, :],
                                            rns[:, t:t + 1])

            # PE-transpose normalized features into matmul operand layout.
            # 4 tile-transposes share one PSUM tile; one engine copy drains
            # them (alternating DVE/Pool).
            def transpose8(src, srcw, dst, ngroups, rows, tag, use_act):
                for g in range(ngroups):
                    pt = psT.tile([rows, 8 * P], BF16, tag=tag)
                    for u in range(8):
                        t = 8 * g + u
                        nc.tensor.transpose(
                            pt[:, u * P:(u + 1) * P],
                            src[:, t * srcw:(t + 1) * srcw], identB[:])
                    dsl = dst[:, g * 8 * P:(g + 1) * 8 * P]
                    if use_act:
                        nc.scalar.activation(dsl, pt[:], AF.Copy)
                    else:
                        nc.vector.tensor_copy(dsl, pt[:])

            transpose8(fnt, D, ftT, MT // 8, D, "pt", False)
            transpose8(fns, D, rhsA, NT // 8, D, "pt", False)

            # -------- prefix-hoist: chunk-0 S/exp for the first HG tiles
            # (these need only ftT/rhsA/srn, not the coord operands, so they
            # fill the PE/ACT window while Pool/DVE build the coord chain)
            HG = 0 if diag else 16
            cols0 = slice(0, CHUNK)
            es0 = []
            for mt in range(HG):
                msl = slice(mt * P, (mt + 1) * P)
                sp = psA.tile([P, CHUNK], F32, tag="sp")
                nc.tensor.matmul(sp[:], ftT[:, msl], rhsA[:, cols0],
                                 start=True, stop=True)
                e = sbE.tile([P, CHUNK], BF16, tag="e")
                nc.scalar.activation(e[:], sp[:], AF.Exp, bias=biasT[:],
                                     scale=srn[:, mt:mt + 1])
                es0.append(e)

            # source: csc = cs - 20, 3-way split; cs2 = |csc|^2, 3-way split
            nc.gpsimd.tensor_scalar_add(csc_all[:], cs_all[:], -CENTER)
            nc.gpsimd.tensor_copy(sh_all[:], csc_all[:])
            nc.gpsimd.tensor_tensor(sr1[:], csc_all[:], sh_all[:],
                                    op=ALU.subtract)
            nc.gpsimd.tensor_copy(sm_all[:], sr1[:])
            nc.gpsimd.tensor_tensor(sl_all[:], sr1[:], sm_all[:],
                                    op=ALU.subtract)
            sqs = scr.tile([P, NT * 3], F32, tag="sqs")
            nc.gpsimd.tensor_tensor(sqs[:], csc_all[:], csc_all[:],
                                    op=ALU.mult)
            nc.vector.tensor_reduce(
                cs2c[:], sqs[:].rearrange("p (t c) -> p t c", c=3),
                axis=mybir.AxisListType.X, op=ALU.add)
            nc.gpsimd.tensor_copy(c2h[:], cs2c[:])
            nc.gpsimd.tensor_tensor(c2r[:], cs2c[:], c2h[:], op=ALU.subtract)
            nc.gpsimd.tensor_copy(c2m[:], c2r[:])
            nc.gpsimd.tensor_tensor(c2l[:], c2r[:], c2m[:], op=ALU.subtract)
            # source rows: [h, cs2h, cs2m, cs2l, m, h, m, l, h]
            rs_ = rbs[:].rearrange("p (t k) -> p t k", k=KW)
            vsh = sh_all[:].rearrange("p (t c) -> p t c", c=3)
            vsm = sm_all[:].rearrange("p (t c) -> p t c", c=3)
            vsl = sl_all[:].rearrange("p (t c) -> p t c", c=3)
            rs2 = rbs[:].rearrange("p (t k) -> p k t", k=KW)
            nc.gpsimd.tensor_copy(rs_[:, :, 0:3], vsh[:])
            nc.gpsimd.tensor_copy(rs2[:, 3, :], c2h[:])
            nc.gpsimd.tensor_copy(rs2[:, 4, :], c2m[:])
            nc.gpsimd.tensor_copy(rs2[:, 5, :], c2l[:])
            nc.gpsimd.tensor_copy(rs_[:, :, 6:9], vsm[:])
            nc.gpsimd.tensor_copy(rs_[:, :, 9:12], vsh[:])
            nc.gpsimd.tensor_copy(rs_[:, :, 12:15], vsm[:])
            nc.gpsimd.tensor_copy(rs_[:, :, 15:18], vsl[:])
            nc.gpsimd.tensor_copy(rs_[:, :, 18:21], vsh[:])
            transpose8(rbs, KW, rhsC, NT // 8, KW, "pt", True)

            # ---------------- coordinates ----------------
            # target: ctn = -2*(ct-20) = -2*ct + 40, 3-way bf16 split
            nc.gpsimd.tensor_scalar(ctn_all[:], ct_all[:], -2.0, 2.0 * CENTER,
                                    op0=ALU.mult, op1=ALU.add)
            nc.gpsimd.tensor_copy(th_all[:], ctn_all[:])
            nc.gpsimd.tensor_tensor(tr1[:], ctn_all[:], th_all[:],
                                    op=ALU.subtract)
            nc.gpsimd.tensor_copy(tm_all[:], tr1[:])
            nc.gpsimd.tensor_tensor(tl_all[:], tr1[:], tm_all[:],
                                    op=ALU.subtract)
            # |ct-20|^2 = |ctn|^2 / 4 ; thr = 100 - |ct-20|^2
            sqc = scr.tile([P, MT * 3], F32, tag="sqc")
            nc.gpsimd.tensor_tensor(sqc[:], ctn_all[:], ctn_all[:],
                                    op=ALU.mult)
            nc.vector.tensor_reduce(
                ct2c[:], sqc[:].rearrange("p (t c) -> p t c", c=3),
                axis=mybir.AxisListType.X, op=ALU.add)
            nc.gpsimd.tensor_scalar(thr[:], ct2c[:], -0.25, RADIUS_SQ,
                                    op0=ALU.mult, op1=ALU.add)
            # agg lhsT: hi/lo split of UNcentered [ct | 1]
            nc.gpsimd.tensor_copy(cth_all[:], ct_all[:])
            nc.gpsimd.tensor_tensor(ctl_all[:], ct_all[:], cth_all[:],
                                    op=ALU.subtract)
            v36 = ct36[:].rearrange("p (t k) -> p t k", k=36)
            vh = cth_all[:].rearrange("p (t c) -> p t c", c=3)
            vl = ctl_all[:].rearrange("p (t c) -> p t c", c=3)
            nc.gpsimd.tensor_copy(v36[:, :, 0:3], vh[:])
            nc.gpsimd.tensor_copy(v36[:, :, 32:35], vl[:])
            # coord lhsT row-layout builder: [h, 1, h, m, m, h, l] then T
            rt = rbt[:].rearrange("p (t k) -> p t k", k=KW)
            vth = th_all[:].rearrange("p (t c) -> p t c", c=3)
            vtm = tm_all[:].rearrange("p (t c) -> p t c", c=3)
            vtl = tl_all[:].rearrange("p (t c) -> p t c", c=3)
            nc.gpsimd.tensor_copy(rt[:, :, 0:3], vth[:])
            nc.gpsimd.tensor_copy(rt[:, :, 6:9], vth[:])
            nc.gpsimd.tensor_copy(rt[:, :, 9:12], vtm[:])
            nc.gpsimd.tensor_copy(rt[:, :, 12:15], vtm[:])
            nc.gpsimd.tensor_copy(rt[:, :, 15:18], vth[:])
            nc.gpsimd.tensor_copy(rt[:, :, 18:21], vtl[:])
            transpose8(rbt, KW, lhsC, MT // 8, KW, "pt", True)

            # ---------------- main loop ----------------
            nchunk_eff = 0 if diag == 'pro' else NCHUNK
            for j in range(nchunk_eff):
                cols = slice(j * CHUNK, (j + 1) * CHUNK)
                aggp = psG.tile([36, CHUNK], F32, tag="agg")
                ks = [None] * MT
                for mt in range(MT):
                    msl = slice(mt * P, (mt + 1) * P)
                    hoisted = (j == 0 and mt < HG)
                    if not hoisted:
                        sp = psA.tile([P, CHUNK], F32, tag="sp")
                        nc.tensor.matmul(sp[:], ftT[:, msl], rhsA[:, cols],
                                         start=True, stop=True)
                    rp = psB.tile([P, CHUNK], F32, tag="rp")
                    nc.tensor.matmul(rp[:], lhsC[0:KC, msl],
                                     rhsC[0:KC, cols], start=True, stop=True)
                    # agg for mt-3 issues here so the PE never waits on the
                    # exp/mask chain of recent mts (3-deep SW pipeline)
                    if mt >= 3:
                        nc.tensor.matmul(
                            aggp[:], ct36[:, 36 * (mt - 3):36 * (mt - 2)],
                            ks[mt - 3][:], start=(mt == 3), stop=False)
                    if hoisted:
                        e = es0[mt]
                    else:
                        e = sbE.tile([P, CHUNK], BF16, tag="e")
                        nc.scalar.activation(e[:], sp[:], AF.Exp,
                                             bias=biasT[:],
                                             scale=srn[:, mt:mt + 1])
                    k = sbK.tile([P, CHUNK], BF16, tag="k")
                    ks[mt] = k
                    nc.vector.scalar_tensor_tensor(k[:], in0=rp[:],
                                             scalar=thr[:, mt:mt + 1],
                                             in1=e[:], op0=ALU.is_lt,
                                             op1=ALU.mult)
                for q in (3, 2, 1):
                    nc.tensor.matmul(
                        aggp[:], ct36[:, 36 * (MT - q):36 * (MT - q + 1)],
                        ks[MT - q][:], start=False, stop=(q == 1))
                agg_hi = fin.tile([4, CHUNK], F32, tag="agghi")
                nc.vector.tensor_copy(agg_hi[:], aggp[0:4, :])
                agg_sb = fin.tile([4, CHUNK], F32, tag="aggsb")
                nc.vector.tensor_tensor(agg_sb[:], agg_hi[:],
                                        aggp[32:36, :], op=ALU.add)
                ntl = CHUNK // P
                tp = psT.tile([P, 4 * ntl], F32, tag="pt")
                for tl in range(ntl):
                    nc.tensor.matmul(tp[:, 4 * tl:4 * tl + 4],
                                     agg_sb[:, tl * P:(tl + 1) * P],
                                     ident[0:4, 0:4], is_transpose=True)
                tsb = fin.tile([P, 4 * ntl], F32, tag="tsb")
                nc.vector.tensor_copy(tsb[:], tp[:])
                tv = tsb[:].rearrange("p (t c) -> p t c", c=4)
                rec = fin.tile([P, ntl], F32, tag="rec")
                nc.vector.tensor_scalar_add(
                    rec[:].rearrange("p (t o) -> p t o", o=1), tv[:, :, 3:4],
                    1e-8)
                nc.vector.reciprocal(rec[:], rec[:])
                outv = out_sb[:].rearrange("p (t c) -> p t c", c=3)
                for tl in range(ntl):
                    nt = j * ntl + tl
                    nc.vector.scalar_tensor_tensor(outv[:, nt, :],
                                                   in0=tv[:, tl, 0:3],
                                                   scalar=rec[:, tl:tl + 1],
                                                   in1=csv[:, nt, :],
                                                   op0=ALU.mult,
                                                   op1=ALU.subtract)
            if nchunk_eff:
                nc.sync.dma_start(
                    out_d.rearrange("(p t) c -> p t c", t=NT),
                    out_sb[:].rearrange("p (t c) -> p t c", c=3))

        if loop_r is None:
            for _rep in range(repeat):
                _body()
        else:
            # Hardware loop: run the identical per-rep program loop_r times
            # in one dispatch (amortized-timing mode for test.py).
            with tc.For_i(0, loop_r, 1):
                _body()

    nc.compile()
    return nc


_CACHE = {}


def kernel(feat_s, feat_t, coor_s, coor_t, epsilon):
    feat_s = np.ascontiguousarray(feat_s, dtype=np.float32)
    feat_t = np.ascontiguousarray(feat_t, dtype=np.float32)
    coor_s = np.ascontiguousarray(coor_s, dtype=np.float32)
    coor_t = np.ascontiguousarray(coor_t, dtype=np.float32)
    tau = float(np.exp(np.float32(epsilon)) + np.float32(TAU_OFFSET))

    key = round(tau, 12)
    if key not in _CACHE:
        _CACHE[key] = build_kernel(tau)
    nc = _CACHE[key]

    in_maps = []
    for c in range(N_CORES):
        b = c // CORES_PER_BATCH
        r = c % CORES_PER_BATCH
        sl = slice(r * NS, (r + 1) * NS)
        in_maps.append({
            "fs": np.ascontiguousarray(feat_s[b, sl]),
            "ft": feat_t[b],
            "cs": np.ascontiguousarray(coor_s[b, sl]),
            "ct": coor_t[b],
        })
    res = run_bass_kernel_spmd(nc, in_maps, core_ids=list(range(N_CORES)))
    out = np.empty((B, N, 3), dtype=np.float32)
    for c in range(N_CORES):
        b = c // CORES_PER_BATCH
        r = c % CORES_PER_BATCH
        out[b, r * NS:(r + 1) * NS] = res.results[c]["out"]
    return out
